# revision 8
# baseline (speedup 1.0000x reference)
"""Multi-head attention (B=4, S=2048, D=2048, H=16) on 8 NeuronCores.

Sharding: 8 cores = 4 batches x 2 head-groups (8 heads each).
Each core:
  - projects its batch's Q/K/V through its head-group's weight slices
  - runs attention for its 8 heads
  - computes a partial output projection (its head-group's contribution)
Host: transposes/casts inputs (free), sums the two partials per batch, adds bo.

On-chip layout avoids all transposes:
  qT, kT = W_slice @ X^T        (host feeds X^T, W^T)
  sT     = kT_chunk.T @ qT      (scores transposed: [kpos, qpos])
  expT   = exp(sT + maskbias)   (ACT, per-partition bias = key mask)
  oT    += v_chunk.T @ expT     (v in natural [S, DK] layout is the lhsT)
  l_rep  = ones.T @ sum(expT)   (column sums replicated across partitions)
  outT   = WoT_chunk.T @ ctxT   (ctxT = normalized oT, overwrites qT buffer)

The v-projection is split by dv-block and interleaved with attention head
groups, so the ACT-heavy (exp) attention work overlaps PE-heavy projection
work instead of serializing behind it.

dtypes: fp16 for X/W/q/k/ctx (precision), bf16 for v/expT (exp range),
fp32 PSUM accumulate everywhere, softmax without max-subtraction
(unscaled scores <= ~60, exp fits fp32; bf16 holds the range).
"""

import numpy as np
from contextlib import ExitStack

import concourse.bass as bass
import concourse.tile as tile
from concourse import bacc, mybir
from concourse.bass_utils import run_bass_kernel_spmd

F16 = mybir.dt.float16
BF16 = mybir.dt.bfloat16
F32 = mybir.dt.float32
P = 128
QB = 512  # qpos / free-dim block size


def build_program(S=2048, D=2048, HG=8, n_cores=8, SKP=None):
    """Build the SPMD per-core program. HG = heads per core (head dim 128).

    SKP = padded compacted-key count (multiple of 128). The host drops
    masked-out key positions (they contribute exactly 0 to the softmax)
    and zero-pads to SKP; pad positions carry maskbias=-1e30 so exp()=0.
    """
    if SKP is None:
        SKP = S
    DK = P
    Dg = HG * DK       # this core's slice of the head dims
    KC = D // P        # contraction chunks over model dim
    SC = S // P        # sequence chunks of 128
    SCK = SKP // P     # compacted key chunks of 128
    NB = S // QB       # sequence blocks of 512
    # key-side blocks of <=512 covering SKP
    kblocks = []
    off = 0
    while off < SKP:
        w = min(QB, SKP - off)
        kblocks.append((off, w))
        off += w
    MQ = HG            # q/k projection output chunks (dq / 128)
    VN = min(QB, Dg)   # v-projection free-dim block
    DVB = Dg // VN
    HPB = VN // DK     # heads covered per v-projection dv-block
    DO = D // P        # out-projection row chunks
    WH = 2             # weight halves (for DMA/SBUF pipelining)
    KCH = KC // WH     # model-dim chunks per weight half
    KOH = max(1, HG // WH)  # wo contraction chunks per half

    nc = bacc.Bacc("TRN2", target_bir_lowering=False, debug=False,
                   num_devices=n_cores)

    # --- I/O ---
    xqT = nc.dram_tensor("xqT", (D, S), F16, kind="ExternalInput").ap()
    xkT = nc.dram_tensor("xkT", (D, SKP), F16, kind="ExternalInput").ap()
    xvT = nc.dram_tensor("xvT", (D, SKP), F16, kind="ExternalInput").ap()
    wqT = nc.dram_tensor("wqT", (D, Dg), F16, kind="ExternalInput").ap()
    wkT = nc.dram_tensor("wkT", (D, Dg), F16, kind="ExternalInput").ap()
    wvT = nc.dram_tensor("wvT", (D, Dg), F16, kind="ExternalInput").ap()
    woT = nc.dram_tensor("woT", (Dg, D), F16, kind="ExternalInput").ap()
    bq_d = nc.dram_tensor("bq", (Dg,), F32, kind="ExternalInput").ap()
    bk_d = nc.dram_tensor("bk", (Dg,), F32, kind="ExternalInput").ap()
    bv_d = nc.dram_tensor("bv_rep", (P, Dg), F32, kind="ExternalInput").ap()
    mb_d = nc.dram_tensor("maskbias", (SKP,), F32, kind="ExternalInput").ap()
    outT = nc.dram_tensor("outT", (D, S), F16, kind="ExternalOutput").ap()

    # --- spill tensors (projection outputs too big for SBUF residency) ---
    kT_sp = nc.dram_tensor("kT_spill", (Dg, SKP), F16, kind="Internal").ap()
    v_sp = nc.dram_tensor("v_spill", (SKP, Dg), BF16, kind="Internal").ap()

    with tile.TileContext(nc) as tc, ExitStack() as ctx:
        consts = ctx.enter_context(tc.tile_pool(name="consts", bufs=1))
        wpool = ctx.enter_context(tc.tile_pool(name="wpool", bufs=3))
        xpool = ctx.enter_context(tc.tile_pool(name="xpool", bufs=24))
        qtpool = ctx.enter_context(tc.tile_pool(name="qtpool", bufs=1))
        hpool = ctx.enter_context(tc.tile_pool(name="hpool", bufs=2))
        epool = ctx.enter_context(tc.tile_pool(name="epool", bufs=14))
        apool = ctx.enter_context(tc.tile_pool(name="apool", bufs=8))
        bpool = ctx.enter_context(tc.tile_pool(name="bpool", bufs=4))
        lpool = ctx.enter_context(tc.tile_pool(name="lpool", bufs=2))
        spool = ctx.enter_context(tc.tile_pool(name="spool", bufs=8))
        psum = ctx.enter_context(tc.tile_pool(name="psum", bufs=1, space="PSUM"))

        # --- constants ---
        bq_sb = consts.tile([P, MQ], F32, name="bq_sb")
        nc.sync.dma_start(bq_sb[:], bq_d.rearrange("(o p) -> p o", p=P))
        bk_sb = consts.tile([P, MQ], F32, name="bk_sb")
        nc.sync.dma_start(bk_sb[:], bk_d.rearrange("(o p) -> p o", p=P))
        bv_sb = consts.tile([P, Dg], F32, name="bv_sb")
        nc.sync.dma_start(bv_sb[:], bv_d)
        mb_sb = consts.tile([P, SCK], F32, name="mb_sb")
        nc.sync.dma_start(mb_sb[:], mb_d.rearrange("(o p) -> p o", p=P))
        ones_sb = consts.tile([P, P], BF16, name="ones_sb")
        nc.any.memset(ones_sb[:], 1.0)

        # resident q^T (later overwritten per-head with normalized o^T = ctx^T)
        qT_sb = qtpool.tile([P, HG, S], F16, name="qT_sb")

        def load_w_halves(w_dram, kchunks, free, label):
            """Load a [kchunks*128, free] weight as WH half-tiles
            [P, kchunks//WH, free]; per-chunk DMAs so matmuls can start as
            soon as their chunk lands (subtile deps)."""
            kh = kchunks // WH
            w_re = w_dram.rearrange("(ko p) m -> p ko m", p=P)
            halves = []
            for i in range(WH):
                t = wpool.tile([P, kh, free], F16, name=f"w_{label}{i}", tag="w")
                for kk in range(kh):
                    nc.sync.dma_start(t[:, kk, :], w_re[:, i * kh + kk, :])
                halves.append(t)
            return halves, kh

        def w_slice(halves, kh, k, lo, hi):
            return halves[k // kh][:, k % kh, lo:hi]

        # ---------------- q/k projections ----------------
        def qk_proj(x_dram, w_dram, bias_sb, label, blocks, evac,
                    interleave_w=False):
            """interleave_w: emit each W k-chunk DMA right before the matching
            X k-chunk DMA of the first block, so the k=0 matmul can start
            after two small DMAs instead of after the whole weight load."""
            if interleave_w:
                kh = KC // WH
                w_re = w_dram.rearrange("(ko p) m -> p ko m", p=P)
                w_halves = [wpool.tile([P, kh, Dg], F16,
                                       name=f"w_{label}{i}", tag="w")
                            for i in range(WH)]
            else:
                w_halves, kh = load_w_halves(w_dram, KC, Dg, label)
            for nb, (off, w) in enumerate(blocks):
                xt = []
                for k in range(KC):
                    if interleave_w and nb == 0:
                        nc.sync.dma_start(
                            w_halves[k // kh][:, k % kh, :], w_re[:, k, :])
                    t = xpool.tile([P, w], F16, name=f"x_{label}_{nb}_{k}",
                                   tag="xs")
                    nc.sync.dma_start(
                        t[:], x_dram[k * P:(k + 1) * P, off:off + w])
                    xt.append(t)
                for m in range(MQ):
                    ps = psum.tile([P, w], F32, name=f"ps_{label}_{nb}_{m}",
                                   tag="psS", bufs=3)
                    for k in range(KC):
                        nc.tensor.matmul(
                            ps[:], w_slice(w_halves, kh, k, m * P, (m + 1) * P),
                            xt[k][:], start=(k == 0), stop=(k == KC - 1))
                    evac(off, w, m, ps, bias_sb)

        def q_evac(off, w, m, ps, bias_sb):
            nc.vector.tensor_tensor(
                qT_sb[:, m, off:off + w], ps[:],
                bias_sb[:, m:m + 1].to_broadcast([P, w]), mybir.AluOpType.add)

        def k_evac(off, w, m, ps, bias_sb):
            st = spool.tile([P, w], F16, name=f"kst_{off}_{m}", tag="st")
            nc.vector.tensor_tensor(
                st[:], ps[:], bias_sb[:, m:m + 1].to_broadcast([P, w]),
                mybir.AluOpType.add)
            nc.gpsimd.dma_start(kT_sp[m * P:(m + 1) * P, off:off + w], st[:])

        qblocks = [(nb * QB, QB) for nb in range(NB)]
        qk_proj(xqT, wqT, bq_sb, "q", qblocks, q_evac, interleave_w=True)
        qk_proj(xkT, wkT, bk_sb, "k", kblocks, k_evac)

        # ---------------- v projection (one dv-block) ----------------
        wv_halves, kvh = load_w_halves(wvT, KC, Dg, "v")

        def v_proj_block(dvb):
            for nb, (off, w) in enumerate(kblocks):
                xt = []
                for k in range(KC):
                    t = xpool.tile([P, w], F16, name=f"x_v{dvb}_{nb}_{k}",
                                   tag="xs")
                    nc.sync.dma_start(
                        t[:], xvT[k * P:(k + 1) * P, off:off + w])
                    xt.append(t)
                for so in range(w // P):
                    s0 = off + so * P
                    ps = psum.tile([P, VN], F32, name=f"ps_v_{dvb}_{nb}_{so}",
                                   tag="psS", bufs=3)
                    for k in range(KC):
                        nc.tensor.matmul(
                            ps[:], xt[k][:, so * P:(so + 1) * P],
                            w_slice(wv_halves, kvh, k, dvb * VN, (dvb + 1) * VN),
                            start=(k == 0), stop=(k == KC - 1))
                    st = spool.tile([P, VN], BF16, name=f"vst_{dvb}_{nb}_{so}",
                                    tag="st")
                    nc.vector.tensor_tensor(
                        st[:], ps[:], bv_sb[:, dvb * VN:(dvb + 1) * VN],
                        mybir.AluOpType.add)
                    nc.gpsimd.dma_start(
                        v_sp[s0:s0 + P, dvb * VN:(dvb + 1) * VN], st[:])

        # ---------------- attention (one head) ----------------
        v_sp_re = v_sp.rearrange("(so p) d -> p so d", p=P)

        def attn_head(h):
            kT_h = hpool.tile([P, SKP], F16, name=f"kT_h{h}", tag="kTh")
            nc.sync.dma_start(kT_h[:], kT_sp[h * P:(h + 1) * P, :])
            v_h = hpool.tile([P, SCK, DK], BF16, name=f"v_h{h}", tag="vh")
            nc.sync.dma_start(v_h[:], v_sp_re[:, :, h * DK:(h + 1) * DK])

            o_ps = [psum.tile([P, QB], F32, name=f"ps_o_{h}_{qb}", tag="psO",
                              bufs=4) for qb in range(NB)]
            acc = [apool.tile([P, QB], F32, name=f"acc_{h}_{qb}", tag="acc")
                   for qb in range(NB)]
            etiles = {}
            for c in range(SCK):
                for qb in range(NB):
                    ps_s = psum.tile([P, QB], F32, name=f"ps_s_{h}_{c}_{qb}",
                                     tag="psS", bufs=3)
                    nc.tensor.matmul(
                        ps_s[:], kT_h[:, c * P:(c + 1) * P],
                        qT_sb[:, h, qb * QB:(qb + 1) * QB],
                        start=True, stop=True)
                    e = epool.tile([P, QB], BF16, name=f"e_{h}_{c}_{qb}",
                                   tag="exp")
                    nc.scalar.activation(
                        e[:], ps_s[:], mybir.ActivationFunctionType.Exp,
                        bias=mb_sb[:, c:c + 1], scale=1.0)
                    etiles[(c, qb)] = e
                    if c == 0:
                        nc.vector.tensor_copy(acc[qb][:], e[:])
                    else:
                        nc.vector.tensor_tensor(acc[qb][:], acc[qb][:], e[:],
                                                mybir.AluOpType.add)
                # AV matmuls pipelined one chunk behind the scores
                if c > 0:
                    for qb in range(NB):
                        nc.tensor.matmul(
                            o_ps[qb][:], v_h[:, c - 1, :],
                            etiles.pop((c - 1, qb))[:],
                            start=(c == 1), stop=False)
            for qb in range(NB):
                nc.tensor.matmul(o_ps[qb][:], v_h[:, SCK - 1, :],
                                 etiles.pop((SCK - 1, qb))[:],
                                 start=(SCK == 1), stop=True)
            for qb in range(NB):
                accb = bpool.tile([P, QB], BF16, name=f"accb_{h}_{qb}",
                                  tag="accb")
                nc.vector.tensor_copy(accb[:], acc[qb][:])
                l_ps = psum.tile([P, QB], F32, name=f"ps_l_{h}_{qb}",
                                 tag="psL", bufs=1)
                nc.tensor.matmul(l_ps[:], ones_sb[:], accb[:],
                                 start=True, stop=True)
                linv = lpool.tile([P, QB], F32, name=f"linv_{h}_{qb}",
                                  tag="linv")
                nc.vector.reciprocal_approx_fast(out=linv[:], in_=l_ps[:])
                # normalized o^T overwrites this head's qT slot (ctx^T)
                nc.vector.tensor_tensor(
                    qT_sb[:, h, qb * QB:(qb + 1) * QB], o_ps[qb][:], linv[:],
                    mybir.AluOpType.mult)

        # v-projection dv-blocks interleaved with their attention heads:
        # PE-heavy projection work overlaps ACT/DVE-heavy attention work.
        for dvb in range(DVB):
            v_proj_block(dvb)
            for h in range(dvb * HPB, (dvb + 1) * HPB):
                attn_head(h)

        # ---------------- output projection ----------------
        wo_re = woT.rearrange("(ko p) m -> p ko m", p=P)
        wo_halves = []
        n_wo_halves = HG // KOH
        for i in range(n_wo_halves):
            t = wpool.tile([P, KOH, D], F16, name=f"w_o{i}", tag="w")
            for kk in range(KOH):
                nc.sync.dma_start(t[:, kk, :], wo_re[:, i * KOH + kk, :])
            wo_halves.append(t)
        for m in range(DO):
            for nb in range(NB):
                ps = psum.tile([P, QB], F32, name=f"ps_out_{m}_{nb}",
                               tag="psO", bufs=4)
                for ko in range(HG):
                    nc.tensor.matmul(
                        ps[:],
                        wo_halves[ko // KOH][:, ko % KOH, m * P:(m + 1) * P],
                        qT_sb[:, ko, nb * QB:(nb + 1) * QB],
                        start=(ko == 0), stop=(ko == HG - 1))
                ost = spool.tile([P, QB], F16, name=f"ost_{m}_{nb}", tag="st")
                nc.vector.tensor_copy(ost[:], ps[:])
                nc.gpsimd.dma_start(
                    outT[m * P:(m + 1) * P, nb * QB:(nb + 1) * QB], ost[:])

    nc.compile()
    return nc


def compute_skp(mask):
    """Padded compacted-key count: max unmasked keys per batch, rounded up
    to a multiple of 128 (at least 128)."""
    mask = np.asarray(mask)
    mx = int((mask != 0).sum(axis=1).max())
    return max(128, (mx + P - 1) // P * P)


def shard_inputs(Q, K, V, mask, Wq, bq, Wk, bk, Wv, bv, Wo, bo,
                 n_cores=8, groups=2, SKP=None):
    """Host-side prep: per-core transposed fp16 inputs. Key positions with
    mask==0 are dropped (exact: they contribute 0 to the softmax); the
    compacted K/V are zero-padded to SKP with maskbias=-1e30 on pads."""
    Q, K, V = (np.asarray(a, np.float32) for a in (Q, K, V))
    mask = np.asarray(mask)
    Wq, Wk, Wv, Wo = (np.asarray(a, np.float32) for a in (Wq, Wk, Wv, Wo))
    bq, bk, bv, bo = (np.asarray(a, np.float32) for a in (bq, bk, bv, bo))
    B, S, D = Q.shape
    Dg = D // groups
    if SKP is None:
        SKP = compute_skp(mask)
    f16 = lambda a: a.astype(np.float16, order="C")
    # per-batch compacted K^T/V^T and mask bias
    xkTs, xvTs, mbs = [], [], []
    for b in range(B):
        idx = np.nonzero(mask[b])[0]
        sk = len(idx)
        xkT = np.zeros((D, SKP), np.float16)
        xkT[:, :sk] = f16(K[b][idx].T)
        xvT = np.zeros((D, SKP), np.float16)
        xvT[:, :sk] = f16(V[b][idx].T)
        mb = np.full(SKP, -1e30, np.float32)
        mb[:sk] = 0.0
        xkTs.append(xkT)
        xvTs.append(xvT)
        mbs.append(mb)
    in_maps = []
    for c in range(n_cores):
        b, g = divmod(c, groups)
        sl = slice(g * Dg, (g + 1) * Dg)
        in_maps.append({
            "xqT": f16(Q[b].T),
            "xkT": xkTs[b],
            "xvT": xvTs[b],
            "wqT": f16(Wq[sl, :].T),
            "wkT": f16(Wk[sl, :].T),
            "wvT": f16(Wv[sl, :].T),
            "woT": f16(Wo[:, sl].T),
            "bq": bq[sl].copy(),
            "bk": bk[sl].copy(),
            "bv_rep": np.tile(bv[sl][None, :], (P, 1)).astype(np.float32),
            "maskbias": mbs[b],
        })
    return in_maps


_PROGRAM_CACHE = {}


def _get_program(S, D, HG, n_cores, SKP):
    key = (S, D, HG, n_cores, SKP)
    if key not in _PROGRAM_CACHE:
        _PROGRAM_CACHE[key] = build_program(S=S, D=D, HG=HG, n_cores=n_cores,
                                            SKP=SKP)
    return _PROGRAM_CACHE[key]


def kernel(Q, K, V, mask, Wq, bq, Wk, bk, Wv, bv, Wo, bo, _trace=False,
           _tmpdir=None):
    Q = np.asarray(Q)
    B, S, D = Q.shape          # 4, 2048, 2048
    n_cores = 8
    groups = n_cores // B      # 2 head-groups
    H_per_group = (D // 128) // groups  # 8 heads per core
    bo = np.asarray(bo, np.float32)

    SKP = compute_skp(mask)
    nc = _get_program(S, D, H_per_group, n_cores, SKP)
    in_maps = shard_inputs(Q, K, V, mask, Wq, bq, Wk, bk, Wv, bv, Wo, bo,
                           n_cores=n_cores, groups=groups, SKP=SKP)
    res = run_bass_kernel_spmd(nc, in_maps, core_ids=list(range(n_cores)),
                               trace=_trace, tmpdir=_tmpdir)
    out = np.empty((B, S, D), np.float32)
    for b in range(B):
        acc = res.results[groups * b]["outT"].astype(np.float32)
        for g in range(1, groups):
            acc += res.results[groups * b + g]["outT"].astype(np.float32)
        out[b] = acc.T + bo[None, :]
    if _trace:
        return out, res
    return out


# revision 9
# speedup vs baseline: 1.0469x; 1.0469x over previous
"""Multi-head attention (B=4, S=2048, D=2048, H=16) on 8 NeuronCores.

Sharding: 8 cores = 4 batches x 2 head-groups (8 heads each).
Each core:
  - projects its batch's Q/K/V through its head-group's weight slices
  - runs attention for its 8 heads
  - computes a partial output projection (its head-group's contribution)
Host: transposes/casts inputs (free), sums the two partials per batch, adds bo.

On-chip layout avoids all transposes:
  qT, kT = W_slice @ X^T        (host feeds X^T, W^T)
  sT     = kT_chunk.T @ qT      (scores transposed: [kpos, qpos])
  expT   = exp(sT + maskbias)   (ACT, per-partition bias = key mask)
  oT    += v_chunk.T @ expT     (v in natural [S, DK] layout is the lhsT)
  l_rep  = ones.T @ sum(expT)   (column sums replicated across partitions)
  outT   = WoT_chunk.T @ ctxT   (ctxT = normalized oT, overwrites qT buffer)

The v-projection is split by dv-block and interleaved with attention head
groups, so the ACT-heavy (exp) attention work overlaps PE-heavy projection
work instead of serializing behind it.

dtypes: fp16 for X/W/q/k/ctx (precision), bf16 for v/expT (exp range),
fp32 PSUM accumulate everywhere, softmax without max-subtraction
(unscaled scores <= ~60, exp fits fp32; bf16 holds the range).
"""

import numpy as np
from contextlib import ExitStack

import concourse.bass as bass
import concourse.tile as tile
from concourse import bacc, mybir
from concourse.bass_utils import run_bass_kernel_spmd

F16 = mybir.dt.float16
BF16 = mybir.dt.bfloat16
F32 = mybir.dt.float32
P = 128
QB = 512  # qpos / free-dim block size


def build_program(S=2048, D=2048, HG=8, n_cores=8, SKP=None):
    """Build the SPMD per-core program. HG = heads per core (head dim 128).

    SKP = padded compacted-key count (multiple of 128). The host drops
    masked-out key positions (they contribute exactly 0 to the softmax)
    and zero-pads to SKP; pad positions carry maskbias=-1e30 so exp()=0.
    """
    if SKP is None:
        SKP = S
    DK = P
    Dg = HG * DK       # this core's slice of the head dims
    KC = D // P        # contraction chunks over model dim
    SC = S // P        # sequence chunks of 128
    SCK = SKP // P     # compacted key chunks of 128
    NB = S // QB       # sequence blocks of 512
    # key-side blocks of <=512 covering SKP
    kblocks = []
    off = 0
    while off < SKP:
        w = min(QB, SKP - off)
        kblocks.append((off, w))
        off += w
    MQ = HG            # q/k projection output chunks (dq / 128)
    VN = min(QB, Dg)   # v-projection free-dim block
    DVB = Dg // VN
    HPB = VN // DK     # heads covered per v-projection dv-block
    DO = D // P        # out-projection row chunks
    WH = 2             # weight halves (for DMA/SBUF pipelining)
    KCH = KC // WH     # model-dim chunks per weight half
    KOH = max(1, HG // WH)  # wo contraction chunks per half

    nc = bacc.Bacc("TRN2", target_bir_lowering=False, debug=False,
                   num_devices=n_cores)

    # --- I/O ---
    xqT = nc.dram_tensor("xqT", (D, S), F16, kind="ExternalInput").ap()
    xkT = nc.dram_tensor("xkT", (D, SKP), F16, kind="ExternalInput").ap()
    xvT = nc.dram_tensor("xvT", (D, SKP), F16, kind="ExternalInput").ap()
    wqT = nc.dram_tensor("wqT", (D, Dg), F16, kind="ExternalInput").ap()
    wkT = nc.dram_tensor("wkT", (D, Dg), F16, kind="ExternalInput").ap()
    wvT = nc.dram_tensor("wvT", (D, Dg), F16, kind="ExternalInput").ap()
    woT = nc.dram_tensor("woT", (Dg, D), F16, kind="ExternalInput").ap()
    bq_d = nc.dram_tensor("bq", (Dg,), F32, kind="ExternalInput").ap()
    bk_d = nc.dram_tensor("bk", (Dg,), F32, kind="ExternalInput").ap()
    bv_d = nc.dram_tensor("bv_rep", (P, Dg), F32, kind="ExternalInput").ap()
    mb_d = nc.dram_tensor("maskbias", (SKP,), F32, kind="ExternalInput").ap()
    outT = nc.dram_tensor("outT", (D, S), F16, kind="ExternalOutput").ap()

    # --- spill tensors (projection outputs too big for SBUF residency) ---
    kT_sp = nc.dram_tensor("kT_spill", (Dg, SKP), F16, kind="Internal").ap()
    v_sp = nc.dram_tensor("v_spill", (SKP, Dg), BF16, kind="Internal").ap()

    with tile.TileContext(nc) as tc, ExitStack() as ctx:
        consts = ctx.enter_context(tc.tile_pool(name="consts", bufs=1))
        wpool = ctx.enter_context(tc.tile_pool(name="wpool", bufs=4))
        xpool = ctx.enter_context(tc.tile_pool(name="xpool", bufs=24))
        qtpool = ctx.enter_context(tc.tile_pool(name="qtpool", bufs=1))
        hpool = ctx.enter_context(tc.tile_pool(name="hpool", bufs=2))
        epool = ctx.enter_context(tc.tile_pool(name="epool", bufs=14))
        apool = ctx.enter_context(tc.tile_pool(name="apool", bufs=8))
        bpool = ctx.enter_context(tc.tile_pool(name="bpool", bufs=4))
        lpool = ctx.enter_context(tc.tile_pool(name="lpool", bufs=2))
        spool = ctx.enter_context(tc.tile_pool(name="spool", bufs=8))
        psum = ctx.enter_context(tc.tile_pool(name="psum", bufs=1, space="PSUM"))

        # --- constants ---
        bq_sb = consts.tile([P, MQ], F32, name="bq_sb")
        nc.sync.dma_start(bq_sb[:], bq_d.rearrange("(o p) -> p o", p=P))
        bk_sb = consts.tile([P, MQ], F32, name="bk_sb")
        nc.sync.dma_start(bk_sb[:], bk_d.rearrange("(o p) -> p o", p=P))
        bv_sb = consts.tile([P, Dg], F32, name="bv_sb")
        nc.sync.dma_start(bv_sb[:], bv_d)
        mb_sb = consts.tile([P, SCK], F32, name="mb_sb")
        nc.sync.dma_start(mb_sb[:], mb_d.rearrange("(o p) -> p o", p=P))
        ones_sb = consts.tile([P, P], BF16, name="ones_sb")
        nc.any.memset(ones_sb[:], 1.0)

        # resident q^T (later overwritten per-head with normalized o^T = ctx^T)
        qT_sb = qtpool.tile([P, HG, S], F16, name="qT_sb")

        def load_w_halves(w_dram, kchunks, free, label):
            """Load a [kchunks*128, free] weight as WH half-tiles
            [P, kchunks//WH, free]; per-chunk DMAs so matmuls can start as
            soon as their chunk lands (subtile deps)."""
            kh = kchunks // WH
            w_re = w_dram.rearrange("(ko p) m -> p ko m", p=P)
            halves = []
            for i in range(WH):
                t = wpool.tile([P, kh, free], F16, name=f"w_{label}{i}", tag="w")
                for kk in range(kh):
                    nc.sync.dma_start(t[:, kk, :], w_re[:, i * kh + kk, :])
                halves.append(t)
            return halves, kh

        def w_slice(halves, kh, k, lo, hi):
            return halves[k // kh][:, k % kh, lo:hi]

        # ---------------- q/k projections ----------------
        def qk_proj(x_dram, w_dram, bias_sb, label, blocks, evac,
                    interleave_w=False):
            """interleave_w: emit each W k-chunk DMA right before the matching
            X k-chunk DMA of the first block, so the k=0 matmul can start
            after two small DMAs instead of after the whole weight load."""
            if interleave_w:
                kh = KC // WH
                w_re = w_dram.rearrange("(ko p) m -> p ko m", p=P)
                w_halves = [wpool.tile([P, kh, Dg], F16,
                                       name=f"w_{label}{i}", tag="w")
                            for i in range(WH)]
            else:
                w_halves, kh = load_w_halves(w_dram, KC, Dg, label)
            for nb, (off, w) in enumerate(blocks):
                xt = []
                for k in range(KC):
                    if interleave_w and nb == 0:
                        nc.sync.dma_start(
                            w_halves[k // kh][:, k % kh, :], w_re[:, k, :])
                    t = xpool.tile([P, w], F16, name=f"x_{label}_{nb}_{k}",
                                   tag="xs")
                    nc.sync.dma_start(
                        t[:], x_dram[k * P:(k + 1) * P, off:off + w])
                    xt.append(t)
                for m in range(MQ):
                    ps = psum.tile([P, w], F32, name=f"ps_{label}_{nb}_{m}",
                                   tag="psS", bufs=3)
                    for k in range(KC):
                        nc.tensor.matmul(
                            ps[:], w_slice(w_halves, kh, k, m * P, (m + 1) * P),
                            xt[k][:], start=(k == 0), stop=(k == KC - 1))
                    evac(off, w, m, ps, bias_sb)

        def q_evac(off, w, m, ps, bias_sb):
            nc.vector.tensor_tensor(
                qT_sb[:, m, off:off + w], ps[:],
                bias_sb[:, m:m + 1].to_broadcast([P, w]), mybir.AluOpType.add)

        def k_evac(off, w, m, ps, bias_sb):
            st = spool.tile([P, w], F16, name=f"kst_{off}_{m}", tag="st")
            nc.vector.tensor_tensor(
                st[:], ps[:], bias_sb[:, m:m + 1].to_broadcast([P, w]),
                mybir.AluOpType.add)
            nc.sync.dma_start(kT_sp[m * P:(m + 1) * P, off:off + w], st[:])

        qblocks = [(nb * QB, QB) for nb in range(NB)]
        qk_proj(xqT, wqT, bq_sb, "q", qblocks, q_evac, interleave_w=True)
        qk_proj(xkT, wkT, bk_sb, "k", kblocks, k_evac)

        # ---------------- v projection (one dv-block) ----------------
        wv_halves, kvh = load_w_halves(wvT, KC, Dg, "v")

        def v_proj_block(dvb):
            for nb, (off, w) in enumerate(kblocks):
                xt = []
                for k in range(KC):
                    t = xpool.tile([P, w], F16, name=f"x_v{dvb}_{nb}_{k}",
                                   tag="xs")
                    nc.sync.dma_start(
                        t[:], xvT[k * P:(k + 1) * P, off:off + w])
                    xt.append(t)
                for so in range(w // P):
                    s0 = off + so * P
                    ps = psum.tile([P, VN], F32, name=f"ps_v_{dvb}_{nb}_{so}",
                                   tag="psS", bufs=3)
                    for k in range(KC):
                        nc.tensor.matmul(
                            ps[:], xt[k][:, so * P:(so + 1) * P],
                            w_slice(wv_halves, kvh, k, dvb * VN, (dvb + 1) * VN),
                            start=(k == 0), stop=(k == KC - 1))
                    st = spool.tile([P, VN], BF16, name=f"vst_{dvb}_{nb}_{so}",
                                    tag="st")
                    nc.vector.tensor_tensor(
                        st[:], ps[:], bv_sb[:, dvb * VN:(dvb + 1) * VN],
                        mybir.AluOpType.add)
                    nc.sync.dma_start(
                        v_sp[s0:s0 + P, dvb * VN:(dvb + 1) * VN], st[:])

        # ---------------- attention (one head) ----------------
        v_sp_re = v_sp.rearrange("(so p) d -> p so d", p=P)

        def attn_head(h):
            kT_h = hpool.tile([P, SKP], F16, name=f"kT_h{h}", tag="kTh")
            nc.sync.dma_start(kT_h[:], kT_sp[h * P:(h + 1) * P, :])
            v_h = hpool.tile([P, SCK, DK], BF16, name=f"v_h{h}", tag="vh")
            nc.sync.dma_start(v_h[:], v_sp_re[:, :, h * DK:(h + 1) * DK])

            o_ps = [psum.tile([P, QB], F32, name=f"ps_o_{h}_{qb}", tag="psO",
                              bufs=4) for qb in range(NB)]
            acc = [apool.tile([P, QB], F32, name=f"acc_{h}_{qb}", tag="acc")
                   for qb in range(NB)]
            etiles = {}
            for c in range(SCK):
                for qb in range(NB):
                    ps_s = psum.tile([P, QB], F32, name=f"ps_s_{h}_{c}_{qb}",
                                     tag="psS", bufs=3)
                    nc.tensor.matmul(
                        ps_s[:], kT_h[:, c * P:(c + 1) * P],
                        qT_sb[:, h, qb * QB:(qb + 1) * QB],
                        start=True, stop=True)
                    e = epool.tile([P, QB], BF16, name=f"e_{h}_{c}_{qb}",
                                   tag="exp")
                    nc.scalar.activation(
                        e[:], ps_s[:], mybir.ActivationFunctionType.Exp,
                        bias=mb_sb[:, c:c + 1], scale=1.0)
                    etiles[(c, qb)] = e
                    if c == 0:
                        nc.vector.tensor_copy(acc[qb][:], e[:])
                    else:
                        nc.vector.tensor_tensor(acc[qb][:], acc[qb][:], e[:],
                                                mybir.AluOpType.add)
                # AV matmuls pipelined one chunk behind the scores
                if c > 0:
                    for qb in range(NB):
                        nc.tensor.matmul(
                            o_ps[qb][:], v_h[:, c - 1, :],
                            etiles.pop((c - 1, qb))[:],
                            start=(c == 1), stop=False)
            for qb in range(NB):
                nc.tensor.matmul(o_ps[qb][:], v_h[:, SCK - 1, :],
                                 etiles.pop((SCK - 1, qb))[:],
                                 start=(SCK == 1), stop=True)
            for qb in range(NB):
                accb = bpool.tile([P, QB], BF16, name=f"accb_{h}_{qb}",
                                  tag="accb")
                nc.vector.tensor_copy(accb[:], acc[qb][:])
                l_ps = psum.tile([P, QB], F32, name=f"ps_l_{h}_{qb}",
                                 tag="psL", bufs=1)
                nc.tensor.matmul(l_ps[:], ones_sb[:], accb[:],
                                 start=True, stop=True)
                linv = lpool.tile([P, QB], F32, name=f"linv_{h}_{qb}",
                                  tag="linv")
                nc.vector.reciprocal_approx_fast(out=linv[:], in_=l_ps[:])
                # normalized o^T overwrites this head's qT slot (ctx^T)
                nc.vector.tensor_tensor(
                    qT_sb[:, h, qb * QB:(qb + 1) * QB], o_ps[qb][:], linv[:],
                    mybir.AluOpType.mult)

        # v-projection dv-blocks interleaved with their attention heads:
        # PE-heavy projection work overlaps ACT/DVE-heavy attention work.
        for dvb in range(DVB):
            v_proj_block(dvb)
            for h in range(dvb * HPB, (dvb + 1) * HPB):
                attn_head(h)

        # ---------------- output projection ----------------
        wo_re = woT.rearrange("(ko p) m -> p ko m", p=P)
        wo_halves = []
        n_wo_halves = HG // KOH
        for i in range(n_wo_halves):
            t = wpool.tile([P, KOH, D], F16, name=f"w_o{i}", tag="w")
            for kk in range(KOH):
                nc.sync.dma_start(t[:, kk, :], wo_re[:, i * KOH + kk, :])
            wo_halves.append(t)
        for m in range(DO):
            for nb in range(NB):
                ps = psum.tile([P, QB], F32, name=f"ps_out_{m}_{nb}",
                               tag="psO", bufs=4)
                for ko in range(HG):
                    nc.tensor.matmul(
                        ps[:],
                        wo_halves[ko // KOH][:, ko % KOH, m * P:(m + 1) * P],
                        qT_sb[:, ko, nb * QB:(nb + 1) * QB],
                        start=(ko == 0), stop=(ko == HG - 1))
                ost = spool.tile([P, QB], F16, name=f"ost_{m}_{nb}", tag="st")
                nc.vector.tensor_copy(ost[:], ps[:])
                nc.sync.dma_start(
                    outT[m * P:(m + 1) * P, nb * QB:(nb + 1) * QB], ost[:])

    nc.compile()
    return nc


def compute_skp(mask):
    """Padded compacted-key count: max unmasked keys per batch, rounded up
    to a multiple of 128 (at least 128)."""
    mask = np.asarray(mask)
    mx = int((mask != 0).sum(axis=1).max())
    return max(128, (mx + P - 1) // P * P)


def shard_inputs(Q, K, V, mask, Wq, bq, Wk, bk, Wv, bv, Wo, bo,
                 n_cores=8, groups=2, SKP=None):
    """Host-side prep: per-core transposed fp16 inputs. Key positions with
    mask==0 are dropped (exact: they contribute 0 to the softmax); the
    compacted K/V are zero-padded to SKP with maskbias=-1e30 on pads."""
    Q, K, V = (np.asarray(a, np.float32) for a in (Q, K, V))
    mask = np.asarray(mask)
    Wq, Wk, Wv, Wo = (np.asarray(a, np.float32) for a in (Wq, Wk, Wv, Wo))
    bq, bk, bv, bo = (np.asarray(a, np.float32) for a in (bq, bk, bv, bo))
    B, S, D = Q.shape
    Dg = D // groups
    if SKP is None:
        SKP = compute_skp(mask)
    f16 = lambda a: a.astype(np.float16, order="C")
    # per-batch compacted K^T/V^T and mask bias
    xkTs, xvTs, mbs = [], [], []
    for b in range(B):
        idx = np.nonzero(mask[b])[0]
        sk = len(idx)
        xkT = np.zeros((D, SKP), np.float16)
        xkT[:, :sk] = f16(K[b][idx].T)
        xvT = np.zeros((D, SKP), np.float16)
        xvT[:, :sk] = f16(V[b][idx].T)
        mb = np.full(SKP, -1e30, np.float32)
        mb[:sk] = 0.0
        xkTs.append(xkT)
        xvTs.append(xvT)
        mbs.append(mb)
    in_maps = []
    for c in range(n_cores):
        b, g = divmod(c, groups)
        sl = slice(g * Dg, (g + 1) * Dg)
        in_maps.append({
            "xqT": f16(Q[b].T),
            "xkT": xkTs[b],
            "xvT": xvTs[b],
            "wqT": f16(Wq[sl, :].T),
            "wkT": f16(Wk[sl, :].T),
            "wvT": f16(Wv[sl, :].T),
            "woT": f16(Wo[:, sl].T),
            "bq": bq[sl].copy(),
            "bk": bk[sl].copy(),
            "bv_rep": np.tile(bv[sl][None, :], (P, 1)).astype(np.float32),
            "maskbias": mbs[b],
        })
    return in_maps


_PROGRAM_CACHE = {}


def _get_program(S, D, HG, n_cores, SKP):
    key = (S, D, HG, n_cores, SKP)
    if key not in _PROGRAM_CACHE:
        _PROGRAM_CACHE[key] = build_program(S=S, D=D, HG=HG, n_cores=n_cores,
                                            SKP=SKP)
    return _PROGRAM_CACHE[key]


def kernel(Q, K, V, mask, Wq, bq, Wk, bk, Wv, bv, Wo, bo, _trace=False,
           _tmpdir=None):
    Q = np.asarray(Q)
    B, S, D = Q.shape          # 4, 2048, 2048
    n_cores = 8
    groups = n_cores // B      # 2 head-groups
    H_per_group = (D // 128) // groups  # 8 heads per core
    bo = np.asarray(bo, np.float32)

    SKP = compute_skp(mask)
    nc = _get_program(S, D, H_per_group, n_cores, SKP)
    in_maps = shard_inputs(Q, K, V, mask, Wq, bq, Wk, bk, Wv, bv, Wo, bo,
                           n_cores=n_cores, groups=groups, SKP=SKP)
    res = run_bass_kernel_spmd(nc, in_maps, core_ids=list(range(n_cores)),
                               trace=_trace, tmpdir=_tmpdir)
    out = np.empty((B, S, D), np.float32)
    for b in range(B):
        acc = res.results[groups * b]["outT"].astype(np.float32)
        for g in range(1, groups):
            acc += res.results[groups * b + g]["outT"].astype(np.float32)
        out[b] = acc.T + bo[None, :]
    if _trace:
        return out, res
    return out
